# revision 6
# baseline (speedup 1.0000x reference)
"""Trainium2 Bass kernel for CategoricalActorCriticNet_HNet_CL.

Math (reference):
    z  = relu(relu(tl @ we1 + be1) @ we2 + be2)                 [B, 64]
    Wa = (relu(z @ aw1 + aw1b) @ aw2 + aw2b).reshape(B, A, S)   hypernet actor head
    ba = relu(z @ ab1 + ab1b) @ ab2 + ab2b
    Wv = (relu(z @ cw1 + cw1b) @ cw2 + cw2b).reshape(B, 1, S)
    bv = relu(z @ cb1 + cb1b) @ cb2 + cb2b
    logits = bmm(Wa, obs) + ba ; v = bmm(Wv, obs) + bv
    log_prob = log_softmax(logits)[b, action[b]] ; entropy

Key identity: Wa is [B, A, S] = 537MB and never needs materializing:
    logits[b,o] = sum_{h,s} aw2[h, o*S+s] * ha[b,h] * obs[b,s] + obs@aw2b_r.T + ba
Per sample b, scale obs by ha[b,h] (a diag matmul on the PE), then contract
with aw2 reshaped [h, s, o].  The h axis (128) is sharded 16-per-core across
8 cores; partial logits are summed with a ReduceScatter which also hands each
core its 32-row batch slice for the softmax/entropy/value epilogue.

Self-contained: hardcodes all shapes; distributes internally over 8 cores.
"""

import os

import numpy as np

import concourse.bass as bass
import concourse.mybir as mybir
import concourse.tile as tile
from concourse.bass_utils import run_bass_kernel_spmd

F32 = mybir.dt.float32
BF16 = mybir.dt.bfloat16
AF = mybir.ActivationFunctionType
ALU = mybir.AluOpType

B = 256          # batch
S = 1024         # state dim
A = 512          # action dim
T = 64           # task dim
E = 64           # embed
H = 128          # hypernet hidden
NC_ = 8          # cores
HC = H // NC_    # h rows per core (16)
BC = B // NC_    # batch rows per core (32)
NSC = S // 128   # state chunks (8)

# matmul weight dtype for the big contraction ("fp32" or "bf16")
MAIN_DT = os.environ.get("BASS_KERNEL_MAIN_DT", "bf16")


def _split_excess_waits(nc, max_waits=1):
    """walrus in this toolchain encodes at most ONE semaphore wait per ISA
    instruction; Tile emits multi-wait instructions (matmuls fed by several
    DMA queues, kernel-tail drains).  Move excess waits onto EventSemaphore
    carriers inserted just before on the same engine queue."""
    n = 0
    for f in nc.m.functions:
        for blk in f.blocks:
            out = []
            for i in blk.instructions:
                si = i.sync_info
                if si is not None and si.on_wait and len(si.on_wait) > max_waits:
                    extra = list(si.on_wait[:-max_waits])
                    for k, w in enumerate(extra):
                        out.append(
                            mybir.InstEventSemaphore(
                                name=f"{i.name}-ws{k}",
                                ins=[],
                                outs=[],
                                engine=i.engine,
                                sync_info=mybir.SyncInfo(on_wait=[w], on_update=[]),
                                bass_nofuse=True,
                            )
                        )
                        n += 1
                    si.on_wait = list(si.on_wait[-max_waits:])
                out.append(i)
            blk.instructions = out
    return n


def _build():
    main_dt = BF16 if MAIN_DT == "bf16" else F32
    nc = bass.Bass("TRN2", num_devices=NC_)

    def inp(name, shape, dtype=F32):
        return nc.dram_tensor(name, shape, dtype, kind="ExternalInput")

    # big matmul weights, per-core h-slice, pre-permuted on host to
    # [h_local, p, sc, o] so each h is one contiguous [128, 4096] DMA
    a_w = inp("a_w", [HC, 128, NSC, A], main_dt)
    obs_in = inp("obs_in", [B, S])              # [b, s] (full batch)
    obs_s = inp("obs_s", [BC, S])               # core's batch slice
    obsTs = inp("obsTs", [128, NSC, BC])        # obs slice folded [p, sc, j]
    tlT = inp("tlT", [T, B])                    # task_label.T
    tlTs = inp("tlTs", [T, BC])                 # slice
    aw2bT = inp("aw2bT", [128, NSC, A])         # aw2b [o*S+s] -> [s,o] folded
    we1 = inp("we1", [T, E])
    be1c = inp("be1c", [E, 1])
    we2 = inp("we2", [E, E])
    be2c = inp("be2c", [E, 1])
    aw1aug = inp("aw1aug", [E + 1, HC])         # [aw1; aw1b] core's h columns
    ab1 = inp("ab1", [E, H])
    ab1bc = inp("ab1bc", [H, 1])
    cw1 = inp("cw1", [E, H])
    cw1bc = inp("cw1bc", [H, 1])
    cb1 = inp("cb1", [E, H])
    cb1bc = inp("cb1bc", [H, 1])
    ab2 = inp("ab2", [H, A])
    ab2br = inp("ab2br", [1, A])
    cw2 = inp("cw2", [H, S])
    cw2br = inp("cw2br", [1, S])
    cb2 = inp("cb2", [H, 1])
    cb2br = inp("cb2br", [1, 1])
    ident = inp("ident", [128, 128])
    act_f = inp("act_f", [BC, 1])

    out_logits = nc.dram_tensor("out_logits", [BC, A], F32, kind="ExternalOutput")
    out_logp = nc.dram_tensor("out_logp", [BC, 1], F32, kind="ExternalOutput")
    out_ent = nc.dram_tensor("out_ent", [BC, 1], F32, kind="ExternalOutput")
    out_v = nc.dram_tensor("out_v", [BC, 1], F32, kind="ExternalOutput")

    with tile.TileContext(nc) as tc:
        with (
            tc.tile_pool(name="const", bufs=1) as cp,
            tc.tile_pool(name="aw", bufs=3) as awp,
            tc.tile_pool(name="scaled", bufs=3) as scp_sb,
            tc.tile_pool(name="small", bufs=1) as sp,
            tc.tile_pool(name="pmain", bufs=1, space="PSUM") as pmain,
            tc.tile_pool(name="pwork", bufs=2, space="PSUM") as pwork,
            tc.tile_pool(name="dram", bufs=1, space="DRAM") as dram,
        ):
            dma = nc.gpsimd.dma_start

            def load(pool, ap, dtype=None, name=None):
                t = pool.tile(ap.shape, dtype or ap.dtype,
                              name=name or f"L_{ap.tensor.name}")
                dma(t[:], ap)
                return t

            # ---- constants into SBUF ----
            obs_sb = [load(cp, obs_in[bt * 128:(bt + 1) * 128, :], name=f"obs{bt}")
                      for bt in range(2)]
            obs_s_sb = load(cp, obs_s[:])
            obsTs_sb = load(cp, obsTs[:])
            tlT_sb = load(cp, tlT[:])
            tlTs_sb = load(cp, tlTs[:])
            aw2bT_sb = load(cp, aw2bT[:])
            we1_sb = load(cp, we1[:])
            be1c_sb = load(cp, be1c[:])
            we2_sb = load(cp, we2[:])
            be2c_sb = load(cp, be2c[:])
            aw1aug_sb = load(cp, aw1aug[:])
            ab1_sb = load(cp, ab1[:])
            ab1bc_sb = load(cp, ab1bc[:])
            cw1_sb = load(cp, cw1[:])
            cw1bc_sb = load(cp, cw1bc[:])
            cb1_sb = load(cp, cb1[:])
            cb1bc_sb = load(cp, cb1bc[:])
            ab2_sb = load(cp, ab2[:])
            ab2br_sb = load(cp, ab2br[:])
            cw2_sb = load(cp, cw2[:])
            cw2br_sb = load(cp, cw2br[:])
            cb2_sb = load(cp, cb2[:])
            cb2br_sb = load(cp, cb2br[:])
            ident_sb = load(cp, ident[:])
            act_f_sb = load(cp, act_f[:])
            ones1 = cp.tile([1, BC], F32)
            nc.vector.memset(ones1[:], 1.0)

            mm = nc.tensor.matmul

            # ---- z-chain, full batch (for ha) ----
            z1p = pwork.tile([E, B], F32, tag="w")
            mm(z1p[:], we1_sb[:], tlT_sb[:], start=True, stop=True)
            z1 = sp.tile([E, B], F32)
            nc.scalar.activation(z1[:], z1p[:], AF.Relu, bias=be1c_sb[:])
            z2p = pwork.tile([E, B], F32, tag="w")
            mm(z2p[:], we2_sb[:], z1[:], start=True, stop=True)
            z2aug = sp.tile([E + 1, B], F32)
            nc.scalar.activation(z2aug[0:E, :], z2p[:], AF.Relu, bias=be2c_sb[:])
            nc.vector.memset(z2aug[E:E + 1, :], 1.0)

            ha = []
            for bt in range(2):
                hap = pwork.tile([128, HC], F32, tag="w")
                mm(hap[:], z2aug[:, bt * 128:(bt + 1) * 128], aw1aug_sb[:],
                   start=True, stop=True)
                h_sb = sp.tile([128, HC], F32, tag=f"ha{bt}")
                nc.scalar.activation(h_sb[:], hap[:], AF.Relu)
                ha.append(h_sb)

            # ---- z-chain, batch slice (for ba / v / bv) ----
            z1ps = pwork.tile([E, BC], F32, tag="w")
            mm(z1ps[:], we1_sb[:], tlTs_sb[:], start=True, stop=True)
            z1s = sp.tile([E, BC], F32)
            nc.scalar.activation(z1s[:], z1ps[:], AF.Relu, bias=be1c_sb[:])
            z2ps = pwork.tile([E, BC], F32, tag="w")
            mm(z2ps[:], we2_sb[:], z1s[:], start=True, stop=True)
            z2s = sp.tile([E, BC], F32)
            nc.scalar.activation(z2s[:], z2ps[:], AF.Relu, bias=be2c_sb[:])

            def hidden_T(w_sb, bias_sb, name):
                hp = pwork.tile([H, BC], F32, tag="w")
                mm(hp[:], w_sb[:], z2s[:], start=True, stop=True)
                h_sb = sp.tile([H, BC], F32, tag=name)
                nc.scalar.activation(h_sb[:], hp[:], AF.Relu, bias=bias_sb[:])
                return h_sb

            hbTs = hidden_T(ab1_sb, ab1bc_sb, "hbTs")   # actor-bias net hidden
            hcTs = hidden_T(cw1_sb, cw1bc_sb, "hcTs")   # critic-W net hidden
            hdTs = hidden_T(cb1_sb, cb1bc_sb, "hdTs")   # critic-bias net hidden

            # ---- main contraction: sum_h sum_s A[h,s,o]*ha[b,h]*obs[b,s] ----
            main_ps = [pmain.tile([128, A], F32, name=f"main{bt}")
                       for bt in range(2)]
            for h in range(HC):
                a_h = awp.tile([128, NSC, A], main_dt, tag="a_h")
                dma(a_h[:], a_w[h])
                for bt in range(2):
                    diag = scp_sb.tile([128, 128], F32, tag="diag")
                    nc.vector.tensor_scalar_mul(
                        diag[:], ident_sb[:], ha[bt][:, h:h + 1])
                    scp = pwork.tile([128, S], F32, tag="w")
                    for sc in range(NSC):
                        mm(scp[:, sc * 128:(sc + 1) * 128],
                           obs_sb[bt][:, sc * 128:(sc + 1) * 128], diag[:],
                           start=True, stop=True)
                    scaled = scp_sb.tile([128, S], main_dt, tag="scaled")
                    nc.any.tensor_copy(scaled[:], scp[:])
                    for sc in range(NSC):
                        mm(main_ps[bt][:], scaled[:, sc * 128:(sc + 1) * 128],
                           a_h[:, sc, :],
                           start=(h == 0 and sc == 0),
                           stop=(h == HC - 1 and sc == NSC - 1),
                           skip_group_check=True)

            # ---- ReduceScatter partial logits; each core gets its b-slice ----
            cc_in = dram.tile([B, A], F32)
            cc_out = dram.tile([BC, A], F32)
            for bt in range(2):
                part = sp.tile([128, A], F32, tag=f"part{bt}")
                nc.any.tensor_copy(part[:], main_ps[bt][:])
                dma(cc_in[bt * 128:(bt + 1) * 128, :], part[:])
            nc.gpsimd.collective_compute(
                "ReduceScatter", ALU.add,
                replica_groups=[list(range(NC_))],
                ins=[cc_in[:].opt()], outs=[cc_out[:].opt()],
            )
            ar = sp.tile([BC, A], F32)
            dma(ar[:], cc_out[:])

            # ---- extras for the slice: c-term obs@aw2b_r.T and ba ----
            extras = pwork.tile([BC, A], F32, tag="w")
            for sc in range(NSC):
                mm(extras[:], obsTs_sb[:, sc, :], aw2bT_sb[:, sc, :],
                   start=(sc == 0), stop=False, skip_group_check=True)
            mm(extras[:], hbTs[:], ab2_sb[:], start=False, stop=False,
               skip_group_check=True)
            mm(extras[:], ones1[:], ab2br_sb[:], start=False, stop=True,
               skip_group_check=True)

            logits = sp.tile([BC, A], F32)
            nc.vector.tensor_add(logits[:], ar[:], extras[:])
            dma(out_logits[:], logits[:])

            # ---- log-softmax / entropy / log_prob ----
            negmax = sp.tile([BC, 1], F32)
            nc.vector.tensor_reduce(negmax[:], logits[:], axis=mybir.AxisListType.X,
                                    op=ALU.max, negate=True)
            e_t = sp.tile([BC, A], F32)
            sumexp = sp.tile([BC, 1], F32)
            nc.scalar.activation(e_t[:], logits[:], AF.Exp, bias=negmax[:],
                                 accum_out=sumexp[:])
            lse = sp.tile([BC, 1], F32)
            nc.scalar.activation(lse[:], sumexp[:], AF.Ln)
            neglogz = sp.tile([BC, 1], F32)
            nc.vector.tensor_sub(neglogz[:], negmax[:], lse[:])  # -(max+lse)
            logp = sp.tile([BC, A], F32)
            nc.scalar.activation(logp[:], logits[:], AF.Identity, bias=neglogz[:])

            # entropy = -sum(e*logp)/sumexp
            junk = sp.tile([BC, A], F32, tag="junk")
            nc.vector.tensor_mul(junk[:], e_t[:], logp[:])
            s_elp = sp.tile([BC, 1], F32)
            nc.vector.tensor_reduce(s_elp[:], junk[:], axis=mybir.AxisListType.X,
                                    op=ALU.add)
            rec = sp.tile([BC, 1], F32)
            nc.vector.reciprocal(rec[:], sumexp[:])
            ent = sp.tile([BC, 1], F32)
            nc.vector.tensor_scalar(ent[:], s_elp[:], rec[:], -1.0,
                                    op0=ALU.mult, op1=ALU.mult)
            dma(out_ent[:], ent[:])

            # log_prob: one-hot(action) dot logp
            iota_t = sp.tile([BC, A], mybir.dt.int32)
            nc.gpsimd.iota(iota_t[:], pattern=[[1, A]], base=0, channel_multiplier=0)
            mask = sp.tile([BC, A], F32)
            nc.vector.tensor_scalar(mask[:], iota_t[:], act_f_sb[:], None,
                                    op0=ALU.is_equal)
            junk2 = sp.tile([BC, A], F32, tag="junk2")
            nc.vector.tensor_mul(junk2[:], mask[:], logp[:])
            lp = sp.tile([BC, 1], F32)
            nc.vector.tensor_reduce(lp[:], junk2[:], axis=mybir.AxisListType.X,
                                    op=ALU.add)
            dma(out_logp[:], lp[:])

            # ---- value head ----
            wv = pwork.tile([BC, S], F32, tag="w")
            for half in range(2):
                sl = slice(half * 512, (half + 1) * 512)
                mm(wv[:, sl], hcTs[:], cw2_sb[:, sl], start=True, stop=False,
                   skip_group_check=True)
                mm(wv[:, sl], ones1[:], cw2br_sb[:, sl], start=False, stop=True,
                   skip_group_check=True)
            junk3 = sp.tile([BC, S], F32, tag="junk3")
            nc.vector.tensor_mul(junk3[:], wv[:], obs_s_sb[:])
            vsum = sp.tile([BC, 1], F32)
            nc.vector.tensor_reduce(vsum[:], junk3[:], axis=mybir.AxisListType.X,
                                    op=ALU.add)
            bvp = pwork.tile([BC, 1], F32, tag="wbv")
            mm(bvp[:], hdTs[:], cb2_sb[:], start=True, stop=False,
               skip_group_check=True)
            mm(bvp[:], ones1[:], cb2br_sb[:], start=False, stop=True,
               skip_group_check=True)
            v_sb = sp.tile([BC, 1], F32)
            nc.vector.tensor_add(v_sb[:], vsum[:], bvp[:])
            dma(out_v[:], v_sb[:])

    _split_excess_waits(nc)
    return nc


_NC_CACHE = {}


def _get_nc():
    key = MAIN_DT
    if key not in _NC_CACHE:
        _NC_CACHE[key] = _build()
    return _NC_CACHE[key]


def _prep_in_maps(inputs):
    f32 = np.float32
    obs = np.ascontiguousarray(inputs["obs"], dtype=f32)
    tl = np.ascontiguousarray(inputs["task_label"], dtype=f32)
    action = np.asarray(inputs["action"])
    aw2 = np.asarray(inputs["aw2"], dtype=f32)

    main_np = np.float32 if MAIN_DT == "fp32" else None
    try:
        import ml_dtypes
        if main_np is None:
            main_np = ml_dtypes.bfloat16
    except ImportError:
        main_np = np.float32

    tlT = np.ascontiguousarray(tl.T)
    aw2bT = np.ascontiguousarray(
        np.asarray(inputs["aw2b"], dtype=f32).reshape(A, S).T
        .reshape(NSC, 128, A).transpose(1, 0, 2))
    aw1aug_full = np.vstack([np.asarray(inputs["aw1"], f32),
                             np.asarray(inputs["aw1b"], f32)[None, :]])
    shared = {
        "obs_in": obs,
        "tlT": tlT,
        "aw2bT": aw2bT,
        "we1": np.asarray(inputs["we1"], f32),
        "be1c": np.asarray(inputs["be1"], f32).reshape(E, 1),
        "we2": np.asarray(inputs["we2"], f32),
        "be2c": np.asarray(inputs["be2"], f32).reshape(E, 1),
        "ab1": np.asarray(inputs["ab1"], f32),
        "ab1bc": np.asarray(inputs["ab1b"], f32).reshape(H, 1),
        "cw1": np.asarray(inputs["cw1"], f32),
        "cw1bc": np.asarray(inputs["cw1b"], f32).reshape(H, 1),
        "cb1": np.asarray(inputs["cb1"], f32),
        "cb1bc": np.asarray(inputs["cb1b"], f32).reshape(H, 1),
        "ab2": np.asarray(inputs["ab2"], f32),
        "ab2br": np.asarray(inputs["ab2b"], f32).reshape(1, A),
        "cw2": np.asarray(inputs["cw2"], f32),
        "cw2br": np.asarray(inputs["cw2b"], f32).reshape(1, S),
        "cb2": np.asarray(inputs["cb2"], f32),
        "cb2br": np.asarray(inputs["cb2b"], f32).reshape(1, 1),
        "ident": np.eye(128, dtype=f32),
    }
    aw2_r = aw2.reshape(H, A, NSC, 128)
    in_maps = []
    for c in range(NC_):
        hsl = slice(c * HC, (c + 1) * HC)
        bsl = slice(c * BC, (c + 1) * BC)
        a_w = np.ascontiguousarray(
            aw2_r[hsl].transpose(0, 3, 2, 1), dtype=main_np)  # [hc, p, sc, o]
        obs_sl = np.ascontiguousarray(obs[bsl])
        obsTs = np.ascontiguousarray(
            obs_sl.T.reshape(NSC, 128, BC).transpose(1, 0, 2))
        m = dict(shared)
        m.update({
            "a_w": a_w,
            "obs_s": obs_sl,
            "obsTs": obsTs,
            "tlTs": np.ascontiguousarray(tl[bsl].T),
            "aw1aug": np.ascontiguousarray(aw1aug_full[:, hsl]),
            "act_f": action.astype(f32).reshape(B, 1)[bsl],
        })
        in_maps.append(m)
    return in_maps, action


def _run(inputs, trace=False):
    nc = _get_nc()
    in_maps, action = _prep_in_maps(inputs)
    res = run_bass_kernel_spmd(nc, in_maps, core_ids=list(range(NC_)),
                               trace=trace)
    logits = np.concatenate([r["out_logits"] for r in res.results], axis=0)
    log_prob = np.concatenate([r["out_logp"] for r in res.results], axis=0)
    entropy = np.concatenate([r["out_ent"] for r in res.results], axis=0)
    v = np.concatenate([r["out_v"] for r in res.results], axis=0)
    return (logits, action, log_prob, entropy, v), res


def kernel(**inputs):
    out, _ = _run(inputs, trace=False)
    return out
